# revision 17
# baseline (speedup 1.0000x reference)
"""BasisLSTMCell Trainium2 kernel (8 NeuronCores, SPMD).

Sharding: 2-way data-parallel over batch x 4-way tensor-parallel over the
units dim.  Core c = g*4 + j handles batch rows [g*1024,(g+1)*1024) and
unit columns [j*256,(j+1)*256) of all four gates.

Math: z[b,n] = sum_{k,u} h[b,u] c[b,k] V[u,k,n]
            + sum_{k,e} x[b,e] c[b,k] W[e,k,n] + bias[n]
The contraction is laid out as r = k*1024+u (64 tiles of 128), then the
x-part r = k*96+e packed into 6 unpadded tiles (each spans two k values;
the host pre-shifts x rows so the on-device build is two aligned
partition-range multiplies), then one bias tile (row 0 = bias, against an
all-ones stationary).  On-device, hcT[r,b] = hT[u,b]*cbT[k,b] tiles are
built on VectorE from host-pretransposed bf16 inputs; TensorE accumulates
fp32 into PSUM; ScalarE+VectorE run the LSTM pointwise.

Each batch group's 8-bank accumulation is split into two 4-bank waves
with wave B lagged LAG contraction tiles behind wave A, so wave A's
pointwise epilogue overlaps the tail of the matmul stream.
"""

import sys

for _p in ("/opt/trn_rl_repo", "/root/.axon_site/_ro/trn_rl_repo"):
    if _p not in sys.path:
        sys.path.insert(0, _p)

import numpy as np
from ml_dtypes import bfloat16

import concourse.bass as bass
import concourse.mybir as mybir
import concourse.tile as tile
from concourse.bass_utils import run_bass_kernel_spmd

B, E, K, U = 2048, 96, 8, 1024
G, J = 2, 4              # batch groups x column groups (G*J = 8 cores)
BC = B // G              # 1024 batch rows per core
NC = 4 * U // J          # 1024 z-columns per core
NTT = 71                 # contraction tiles: 64 recurrent + 6 x + 1 bias
LAG = 8                  # wave-B lag in contraction tiles
BF = mybir.dt.bfloat16
F32 = mybir.dt.float32
AF = mybir.ActivationFunctionType

_cache = {}


def _split_excess_waits(nc, max_waits=1):
    """Walrus CoreV3 codegen accepts at most one sync-wait command per
    instruction; Tile's final drain can carry more.  Move the excess onto
    preceding same-engine NoOps (the engine executes in order, so the
    chain is semantically identical)."""
    for f in nc.m.functions:
        for bb in f.blocks:
            insts = list(bb.instructions)
            changed = False
            new = []
            for inst in insts:
                si = inst.sync_info
                if si is not None and len(si.on_wait) > max_waits:
                    waits = list(si.on_wait)
                    extra, keep = waits[:-max_waits], waits[-max_waits:]
                    for j in range(0, len(extra), max_waits):
                        new.append(mybir.InstNoOp(
                            name=f"{inst.name}-wsplit{j}",
                            engine=inst.engine,
                            sync_info=mybir.SyncInfo(
                                on_wait=extra[j:j + max_waits], on_update=[]),
                            bass_nofuse=True,
                        ))
                    inst.sync_info = mybir.SyncInfo(
                        on_wait=keep, on_update=si.on_update)
                    changed = True
                new.append(inst)
            if changed:
                while len(bb.instructions):
                    bb.instructions.pop()
                for i in new:
                    bb.instructions.append(i)


def _build():
    nc = bass.Bass("TRN2", target_bir_lowering=False, debug=False,
                   num_devices=G * J)
    hT_d = nc.dram_tensor("hT", [8, 128, BC], BF, kind="ExternalInput").ap()
    xTp_d = nc.dram_tensor("xTp", [6, 128, BC], BF, kind="ExternalInput").ap()
    xcb_d = nc.dram_tensor("xcb", [6, 128, BC], BF, kind="ExternalInput").ap()
    cbT_d = nc.dram_tensor("cbT", [8, 128, BC], BF, kind="ExternalInput").ap()
    Vw_d = nc.dram_tensor("Vw", [NTT, 128, NC], BF,
                          kind="ExternalInput").ap()
    cold_d = nc.dram_tensor("c_tm1", [BC, 256], F32,
                            kind="ExternalInput").ap()
    h_out_d = nc.dram_tensor("h_out", [BC, 256], F32,
                             kind="ExternalOutput").ap()
    c_out_d = nc.dram_tensor("c_out", [BC, 256], F32,
                             kind="ExternalOutput").ap()

    with tile.TileContext(nc) as tc:
        with tc.tile_pool(name="src", bufs=1) as srcp, \
             tc.tile_pool(name="w", bufs=18) as wp, \
             tc.tile_pool(name="hc", bufs=LAG + 4) as hcp, \
             tc.tile_pool(name="pw", bufs=4) as pwp, \
             tc.tile_pool(name="io", bufs=4) as iop, \
             tc.tile_pool(name="psum", bufs=1, space="PSUM") as psp:

            ones_t = srcp.tile([128, 512], BF, tag="ones", name="ones")
            nc.vector.memset(ones_t[:, :], 1.0)

            # HAM warmup: ~7us of tiny matmuls so the PE clock is at 8/8
            # by the time the real stream starts (shares bank q0).
            warm = psp.tile([128, 64], F32, tag="q0", name="warm")
            for _ in range(200):
                nc.tensor.matmul(warm[:, :], ones_t[:, 0:128],
                                 ones_t[:, 0:64], start=True, stop=True)

            hT_t = [None] * 8
            cb_t = [None] * 8
            # load order matches first use: t=0..7 need hT0..7 + cb0
            t_ = srcp.tile([128, BC], BF, tag="hT0", name="hT0")
            nc.sync.dma_start(out=t_[:, :], in_=hT_d[0])
            hT_t[0] = t_
            t_ = srcp.tile([128, BC], BF, tag="cb0", name="cb0")
            nc.sync.dma_start(out=t_[:, :], in_=cbT_d[0])
            cb_t[0] = t_
            for i in range(1, 8):
                t_ = srcp.tile([128, BC], BF, tag=f"hT{i}", name=f"hT{i}")
                nc.sync.dma_start(out=t_[:, :], in_=hT_d[i])
                hT_t[i] = t_
            for k in range(1, 8):
                t_ = srcp.tile([128, BC], BF, tag=f"cb{k}", name=f"cb{k}")
                nc.sync.dma_start(out=t_[:, :], in_=cbT_d[k])
                cb_t[k] = t_
            # x-part tiles are first needed at t=64; loaded lazily (below)
            # to keep startup HBM bandwidth for hT/cb/weights
            xp_t = [None] * 6
            xcb_t = [None] * 6

            def load_x_tiles():
                for xt in range(6):
                    t_ = srcp.tile([128, BC], BF, tag=f"xp{xt}",
                                   name=f"xp{xt}")
                    nc.gpsimd.dma_start(out=t_[:, :], in_=xTp_d[xt])
                    xp_t[xt] = t_
                    t_ = srcp.tile([128, BC], BF, tag=f"xc{xt}",
                                   name=f"xc{xt}")
                    nc.gpsimd.dma_start(out=t_[:, :], in_=xcb_d[xt])
                    xcb_t[xt] = t_

            def pointwise(bank, mloc_pair, mg):
                # bank[q]: q = nh*4 + mloc; p0 = z[:,0:512] (i|f),
                # p1 = z[:,512:1024] (g|o) for m-tile mloc
                for mloc in mloc_pair:
                    row0 = mg * 512 + mloc * 128
                    p0, p1 = bank[mloc], bank[4 + mloc]
                    co = iop.tile([128, 256], F32, tag="co", name="cot")
                    nc.sync.dma_start(out=co[:, :],
                                      in_=cold_d[row0:row0 + 128, :])
                    if_s = pwp.tile([128, 512], F32, tag="if", name="ift")
                    nc.scalar.activation(if_s[:, :], p0[:, :], AF.Sigmoid)
                    g_s = pwp.tile([128, 256], F32, tag="g", name="gt")
                    nc.scalar.activation(g_s[:, :], p1[:, 0:256], AF.Tanh)
                    o_s = pwp.tile([128, 256], F32, tag="o", name="ot")
                    nc.scalar.activation(o_s[:, :], p1[:, 256:512],
                                         AF.Sigmoid)
                    t1 = pwp.tile([128, 256], F32, tag="t1", name="t1t")
                    nc.vector.tensor_mul(t1[:, :], if_s[:, 0:256], g_s[:, :])
                    t2 = pwp.tile([128, 256], F32, tag="t2", name="t2t")
                    nc.vector.tensor_mul(t2[:, :], if_s[:, 256:512],
                                         co[:, :])
                    c_n = iop.tile([128, 256], F32, tag="cn", name="cnt")
                    nc.vector.tensor_add(c_n[:, :], t1[:, :], t2[:, :])
                    th = pwp.tile([128, 256], F32, tag="th", name="tht")
                    nc.scalar.activation(th[:, :], c_n[:, :], AF.Tanh)
                    h_n = iop.tile([128, 256], F32, tag="hn", name="hnt")
                    nc.vector.tensor_mul(h_n[:, :], o_s[:, :], th[:, :])
                    nc.sync.dma_start(out=h_out_d[row0:row0 + 128, :],
                                      in_=h_n[:, :])
                    nc.sync.dma_start(out=c_out_d[row0:row0 + 128, :],
                                      in_=c_n[:, :])

            for mg in range(2):           # groups of 4 batch tiles (512 rows)
                bsl = slice(mg * 512, (mg + 1) * 512)
                bank = [psp.tile([128, 512], F32, tag=f"q{q}", name=f"q{q}")
                        for q in range(8)]
                w_tiles = {}
                hc_tiles = {}

                def mm(t, mlocs, bank=bank, w_tiles=w_tiles,
                       hc_tiles=hc_tiles):
                    hc, w_t = hc_tiles[t], w_tiles[t]
                    for mloc in mlocs:
                        for nh in range(2):
                            nc.tensor.matmul(
                                bank[nh * 4 + mloc][:, :],
                                hc[:, mloc * 128:(mloc + 1) * 128],
                                w_t[:, nh * 512:(nh + 1) * 512],
                                start=(t == 0), stop=(t == NTT - 1))

                for s in range(NTT + LAG):
                    if mg == 0 and s == 24:
                        load_x_tiles()
                    if s < NTT:
                        t = s
                        w_t = wp.tile([128, NC], BF, tag="w", name="wt")
                        # gpsimd queue: don't serialize behind src loads
                        nc.gpsimd.dma_start(out=w_t[:, :], in_=Vw_d[t])
                        w_tiles[t] = w_t
                        if t < 64:
                            hc = hcp.tile([128, 512], BF, tag="hc",
                                          name="hct")
                            nc.vector.tensor_mul(
                                hc[:, :], hT_t[t % 8][:, bsl],
                                cb_t[t // 8][:, bsl])
                        elif t < 70:
                            xt = t - 64
                            hc = hcp.tile([128, 512], BF, tag="hc",
                                          name="hct")
                            nc.vector.tensor_mul(
                                hc[:, :], xp_t[xt][:, bsl],
                                xcb_t[xt][:, bsl])
                        else:
                            hc = ones_t
                        hc_tiles[t] = hc
                        mm(t, (0, 1))          # wave A
                    if 0 <= s - LAG:
                        mm(s - LAG, (2, 3))    # wave B
                        del hc_tiles[s - LAG]
                        del w_tiles[s - LAG]
                    if s == NTT - 1:
                        pointwise(bank, (0, 1), mg)   # wave A epilogue
                pointwise(bank, (2, 3), mg)           # wave B epilogue

    _split_excess_waits(nc)
    return nc


def _prep_in_maps(inputs, h_tm1, c_tm1, basis_kernel, basis_recurrent_kernel,
                  bias):
    x = np.asarray(inputs[:, :E], np.float32)
    c_prob = np.asarray(inputs[:, E:], np.float32)
    h_tm1 = np.asarray(h_tm1, np.float32)
    c_tm1 = np.asarray(c_tm1, np.float32)

    Vr = np.asarray(basis_recurrent_kernel, np.float32) \
        .transpose(1, 0, 2).reshape(K * U, 4 * U)
    Wx = np.asarray(basis_kernel, np.float32) \
        .transpose(1, 0, 2).reshape(K * E, 4 * U)
    Bt = np.zeros((128, 4 * U), np.float32)
    Bt[0] = np.asarray(bias, np.float32)
    Vw_full = np.concatenate([Vr, Wx, Bt], 0)    # [NTT*128, 4096]

    er = np.arange(K * E) % E                    # pre-shifted x row -> e
    kr = np.arange(K * E) // E                   # pre-shifted x row -> k

    in_maps = []
    for g in range(G):
        bsl = slice(g * BC, (g + 1) * BC)
        hT = np.ascontiguousarray(h_tm1[bsl].T).astype(bfloat16) \
            .reshape(8, 128, BC)
        xTp = np.ascontiguousarray(x[bsl].T[er]).astype(bfloat16) \
            .reshape(6, 128, BC)
        xcb = np.ascontiguousarray(c_prob[bsl].T[kr]).astype(bfloat16) \
            .reshape(6, 128, BC)
        cbT = np.ascontiguousarray(
            np.broadcast_to(c_prob[bsl].T[:, None, :], (8, 128, BC))
        ).astype(bfloat16)
        for j in range(J):
            cols = np.concatenate(
                [np.arange(gt * U + j * 256, gt * U + (j + 1) * 256)
                 for gt in range(4)])
            Vw_c = np.ascontiguousarray(Vw_full[:, cols]).astype(bfloat16) \
                .reshape(NTT, 128, NC)
            co = np.ascontiguousarray(c_tm1[bsl, j * 256:(j + 1) * 256])
            in_maps.append({"hT": hT, "xTp": xTp, "xcb": xcb, "cbT": cbT,
                            "Vw": Vw_c, "c_tm1": co})
    return in_maps


def _run(in_maps, trace=False, **kw):
    if "nc" not in _cache:
        _cache["nc"] = _build()
    return run_bass_kernel_spmd(_cache["nc"], in_maps,
                                list(range(G * J)), trace=trace, **kw)


def kernel(inputs, h_tm1, c_tm1, basis_kernel, basis_recurrent_kernel, bias,
           _trace=False, **_kw):
    in_maps = _prep_in_maps(inputs, h_tm1, c_tm1, basis_kernel,
                            basis_recurrent_kernel, bias)
    res = _run(in_maps, trace=_trace, **_kw)
    h = np.empty((B, U), np.float32)
    c = np.empty((B, U), np.float32)
    for g in range(G):
        for j in range(J):
            r = res.results[g * J + j]
            h[g * BC:(g + 1) * BC, j * 256:(j + 1) * 256] = r["h_out"]
            c[g * BC:(g + 1) * BC, j * 256:(j + 1) * 256] = r["c_out"]
    kernel.last_results = res
    return (h, c)


# revision 25
# speedup vs baseline: 1.0024x; 1.0024x over previous
"""BasisLSTMCell Trainium2 kernel (8 NeuronCores, SPMD).

Sharding: 2-way data-parallel over batch x 4-way tensor-parallel over the
units dim.  Core c = g*4 + j handles batch rows [g*1024,(g+1)*1024) and
unit columns [j*256,(j+1)*256) of all four gates.

Math: z[b,n] = sum_{k,u} h[b,u] c[b,k] V[u,k,n]
            + sum_{k,e} x[b,e] c[b,k] W[e,k,n] + bias[n]
The contraction is laid out as r = k*1024+u (64 tiles of 128), then the
x-part r = k*96+e packed into 6 unpadded tiles (each spans two k values;
the host pre-shifts x rows so the on-device build is two aligned
partition-range multiplies), then one bias tile (row 0 = bias, against an
all-ones stationary).  On-device, hcT[r,b] = hT[u,b]*cbT[k,b] tiles are
built on VectorE from host-pretransposed bf16 inputs; TensorE accumulates
fp32 into PSUM; ScalarE+VectorE run the LSTM pointwise.

Each batch group's 8-bank accumulation is split into two 4-bank waves
with wave B lagged LAG contraction tiles behind wave A, so wave A's
pointwise epilogue overlaps the tail of the matmul stream.
"""

import sys

for _p in ("/opt/trn_rl_repo", "/root/.axon_site/_ro/trn_rl_repo"):
    if _p not in sys.path:
        sys.path.insert(0, _p)

import numpy as np
from ml_dtypes import bfloat16

import concourse.bass as bass
import concourse.mybir as mybir
import concourse.tile as tile
from concourse.bass_utils import run_bass_kernel_spmd

B, E, K, U = 2048, 96, 8, 1024
G, J = 2, 4              # batch groups x column groups (G*J = 8 cores)
BC = B // G              # 1024 batch rows per core
NC = 4 * U // J          # 1024 z-columns per core
NTT = 71                 # contraction tiles: 64 recurrent + 6 x + 1 bias
LAG = 16                 # wave-B lag in contraction tiles
BF = mybir.dt.bfloat16
F32 = mybir.dt.float32
AF = mybir.ActivationFunctionType

_cache = {}


def _split_excess_waits(nc, max_waits=1):
    """Walrus CoreV3 codegen accepts at most one sync-wait command per
    instruction; Tile's final drain can carry more.  Move the excess onto
    preceding same-engine NoOps (the engine executes in order, so the
    chain is semantically identical)."""
    for f in nc.m.functions:
        for bb in f.blocks:
            insts = list(bb.instructions)
            changed = False
            new = []
            for inst in insts:
                si = inst.sync_info
                if si is not None and len(si.on_wait) > max_waits:
                    waits = list(si.on_wait)
                    extra, keep = waits[:-max_waits], waits[-max_waits:]
                    for j in range(0, len(extra), max_waits):
                        new.append(mybir.InstNoOp(
                            name=f"{inst.name}-wsplit{j}",
                            engine=inst.engine,
                            sync_info=mybir.SyncInfo(
                                on_wait=extra[j:j + max_waits], on_update=[]),
                            bass_nofuse=True,
                        ))
                    inst.sync_info = mybir.SyncInfo(
                        on_wait=keep, on_update=si.on_update)
                    changed = True
                new.append(inst)
            if changed:
                while len(bb.instructions):
                    bb.instructions.pop()
                for i in new:
                    bb.instructions.append(i)


def _build():
    nc = bass.Bass("TRN2", target_bir_lowering=False, debug=False,
                   num_devices=G * J)
    hT_d = nc.dram_tensor("hT", [8, 128, BC], BF, kind="ExternalInput").ap()
    xTp_d = nc.dram_tensor("xTp", [6, 128, BC], BF, kind="ExternalInput").ap()
    xcb_d = nc.dram_tensor("xcb", [6, 128, BC], BF, kind="ExternalInput").ap()
    cbT_d = nc.dram_tensor("cbT", [8, 128, BC], BF, kind="ExternalInput").ap()
    Vw_d = nc.dram_tensor("Vw", [NTT, 128, NC], BF,
                          kind="ExternalInput").ap()
    cold_d = nc.dram_tensor("c_tm1", [BC, 256], F32,
                            kind="ExternalInput").ap()
    h_out_d = nc.dram_tensor("h_out", [BC, 256], F32,
                             kind="ExternalOutput").ap()
    c_out_d = nc.dram_tensor("c_out", [BC, 256], F32,
                             kind="ExternalOutput").ap()

    with tile.TileContext(nc) as tc:
        with tc.tile_pool(name="src", bufs=1) as srcp, \
             tc.tile_pool(name="w", bufs=LAG + 6) as wp, \
             tc.tile_pool(name="hc", bufs=LAG + 4) as hcp, \
             tc.tile_pool(name="pw", bufs=4) as pwp, \
             tc.tile_pool(name="io", bufs=4) as iop, \
             tc.tile_pool(name="psum", bufs=1, space="PSUM") as psp:

            ones_t = srcp.tile([128, 512], BF, tag="ones", name="ones")
            nc.vector.memset(ones_t[:, :], 1.0)

            # HAM warmup: ~6us of N=512 matmuls so the PE clock is at 8/8
            # by the time the real stream starts (shares bank q0).
            warm = psp.tile([128, 512], F32, tag="q0", name="warm")
            for _ in range(16):
                nc.tensor.matmul(warm[:, :], ones_t[:, 0:128],
                                 ones_t[:, 0:512], start=True, stop=True)

            hT_t = [None] * 8
            cb_t = [None] * 8
            # load order matches first use: t=0..7 need hT0..7 + cb0
            t_ = srcp.tile([128, BC], BF, tag="hT0", name="hT0")
            nc.sync.dma_start(out=t_[:, :], in_=hT_d[0])
            hT_t[0] = t_
            t_ = srcp.tile([128, BC], BF, tag="cb0", name="cb0")
            nc.sync.dma_start(out=t_[:, :], in_=cbT_d[0])
            cb_t[0] = t_
            for i in range(1, 8):
                t_ = srcp.tile([128, BC], BF, tag=f"hT{i}", name=f"hT{i}")
                nc.sync.dma_start(out=t_[:, :], in_=hT_d[i])
                hT_t[i] = t_
            for k in range(1, 8):
                t_ = srcp.tile([128, BC], BF, tag=f"cb{k}", name=f"cb{k}")
                nc.sync.dma_start(out=t_[:, :], in_=cbT_d[k])
                cb_t[k] = t_
            xp_t = []
            xcb_t = []
            for xt in range(6):
                t_ = srcp.tile([128, BC], BF, tag=f"xp{xt}", name=f"xp{xt}")
                nc.sync.dma_start(out=t_[:, :], in_=xTp_d[xt])
                xp_t.append(t_)
                t_ = srcp.tile([128, BC], BF, tag=f"xc{xt}", name=f"xc{xt}")
                nc.sync.dma_start(out=t_[:, :], in_=xcb_d[xt])
                xcb_t.append(t_)

            def pointwise(bank, mloc_pair, mg):
                # bank[q]: q = nh*4 + mloc; p0 = z[:,0:512] (i|f),
                # p1 = z[:,512:1024] (g|o) for m-tile mloc
                for mloc in mloc_pair:
                    row0 = mg * 512 + mloc * 128
                    p0, p1 = bank[mloc], bank[4 + mloc]
                    co = iop.tile([128, 256], F32, tag="co", name="cot")
                    nc.sync.dma_start(out=co[:, :],
                                      in_=cold_d[row0:row0 + 128, :])
                    if_s = pwp.tile([128, 512], F32, tag="if", name="ift")
                    nc.scalar.activation(if_s[:, :], p0[:, :], AF.Sigmoid)
                    g_s = pwp.tile([128, 256], F32, tag="g", name="gt")
                    nc.scalar.activation(g_s[:, :], p1[:, 0:256], AF.Tanh)
                    o_s = pwp.tile([128, 256], F32, tag="o", name="ot")
                    nc.scalar.activation(o_s[:, :], p1[:, 256:512],
                                         AF.Sigmoid)
                    t1 = pwp.tile([128, 256], F32, tag="t1", name="t1t")
                    nc.vector.tensor_mul(t1[:, :], if_s[:, 0:256], g_s[:, :])
                    t2 = pwp.tile([128, 256], F32, tag="t2", name="t2t")
                    nc.vector.tensor_mul(t2[:, :], if_s[:, 256:512],
                                         co[:, :])
                    c_n = iop.tile([128, 256], F32, tag="cn", name="cnt")
                    nc.vector.tensor_add(c_n[:, :], t1[:, :], t2[:, :])
                    th = pwp.tile([128, 256], F32, tag="th", name="tht")
                    nc.scalar.activation(th[:, :], c_n[:, :], AF.Tanh)
                    h_n = iop.tile([128, 256], F32, tag="hn", name="hnt")
                    nc.vector.tensor_mul(h_n[:, :], o_s[:, :], th[:, :])
                    nc.sync.dma_start(out=h_out_d[row0:row0 + 128, :],
                                      in_=h_n[:, :])
                    nc.sync.dma_start(out=c_out_d[row0:row0 + 128, :],
                                      in_=c_n[:, :])

            for mg in range(2):           # groups of 4 batch tiles (512 rows)
                bsl = slice(mg * 512, (mg + 1) * 512)
                bank = [psp.tile([128, 512], F32, tag=f"q{q}", name=f"q{q}")
                        for q in range(8)]
                w_tiles = {}
                hc_tiles = {}

                def mm(t, mlocs, bank=bank, w_tiles=w_tiles,
                       hc_tiles=hc_tiles):
                    hc, w_t = hc_tiles[t], w_tiles[t]
                    for mloc in mlocs:
                        for nh in range(2):
                            nc.tensor.matmul(
                                bank[nh * 4 + mloc][:, :],
                                hc[:, mloc * 128:(mloc + 1) * 128],
                                w_t[:, nh * 512:(nh + 1) * 512],
                                start=(t == 0), stop=(t == NTT - 1))

                # mg0 runs all 8 banks in lockstep (weight-DMA demand at
                # startup stays at one tile per 8 matmuls); mg1 runs the
                # lagged two-wave schedule so its wave-A pointwise overlaps
                # the matmul tail (its weights prefetch during mg0).
                lag = 0 if mg == 0 else LAG
                for s in range(NTT + lag):
                    if s < NTT:
                        t = s
                        w_t = wp.tile([128, NC], BF, tag="w", name="wt")
                        # alternate queues; don't serialize behind src loads
                        eng = nc.gpsimd if t % 2 == 0 else nc.scalar
                        eng.dma_start(out=w_t[:, :], in_=Vw_d[t])
                        w_tiles[t] = w_t
                        if t < 64:
                            hc = hcp.tile([128, 512], BF, tag="hc",
                                          name="hct")
                            nc.vector.tensor_mul(
                                hc[:, :], hT_t[t % 8][:, bsl],
                                cb_t[t // 8][:, bsl])
                        elif t < 70:
                            xt = t - 64
                            hc = hcp.tile([128, 512], BF, tag="hc",
                                          name="hct")
                            nc.vector.tensor_mul(
                                hc[:, :], xp_t[xt][:, bsl],
                                xcb_t[xt][:, bsl])
                        else:
                            hc = ones_t
                        hc_tiles[t] = hc
                        mm(t, (0, 1, 2))       # wave A
                        if lag == 0:
                            mm(t, (3,))
                    if lag and 0 <= s - lag:
                        mm(s - lag, (3,))      # wave B
                        del hc_tiles[s - lag]
                        del w_tiles[s - lag]
                    if s == NTT - 1:
                        pointwise(bank, (0, 1, 2), mg)   # wave A epilogue
                pointwise(bank, (3,), mg)                # wave B epilogue

    _split_excess_waits(nc)
    return nc


def _prep_in_maps(inputs, h_tm1, c_tm1, basis_kernel, basis_recurrent_kernel,
                  bias):
    inputs = np.asarray(inputs, np.float32)
    x = inputs[:, :E]
    c_prob = inputs[:, E:]
    h_tm1 = np.asarray(h_tm1, np.float32)
    c_tm1 = np.asarray(c_tm1, np.float32)

    Vr = np.asarray(basis_recurrent_kernel, np.float32) \
        .transpose(1, 0, 2).reshape(K * U, 4 * U)
    Wx = np.asarray(basis_kernel, np.float32) \
        .transpose(1, 0, 2).reshape(K * E, 4 * U)
    Bt = np.zeros((128, 4 * U), np.float32)
    Bt[0] = np.asarray(bias, np.float32)
    Vw_full = np.concatenate([Vr, Wx, Bt], 0)    # [NTT*128, 4096]

    er = np.arange(K * E) % E                    # pre-shifted x row -> e
    kr = np.arange(K * E) // E                   # pre-shifted x row -> k

    in_maps = []
    for g in range(G):
        bsl = slice(g * BC, (g + 1) * BC)
        hT = np.ascontiguousarray(h_tm1[bsl].T).astype(bfloat16) \
            .reshape(8, 128, BC)
        xTp = np.ascontiguousarray(x[bsl].T[er]).astype(bfloat16) \
            .reshape(6, 128, BC)
        xcb = np.ascontiguousarray(c_prob[bsl].T[kr]).astype(bfloat16) \
            .reshape(6, 128, BC)
        cbT = np.ascontiguousarray(
            np.broadcast_to(c_prob[bsl].T[:, None, :], (8, 128, BC))
        ).astype(bfloat16)
        for j in range(J):
            cols = np.concatenate(
                [np.arange(gt * U + j * 256, gt * U + (j + 1) * 256)
                 for gt in range(4)])
            Vw_c = np.ascontiguousarray(Vw_full[:, cols]).astype(bfloat16) \
                .reshape(NTT, 128, NC)
            co = np.ascontiguousarray(c_tm1[bsl, j * 256:(j + 1) * 256])
            in_maps.append({"hT": hT, "xTp": xTp, "xcb": xcb, "cbT": cbT,
                            "Vw": Vw_c, "c_tm1": co})
    return in_maps


def _run(in_maps, trace=False, **kw):
    if "nc" not in _cache:
        _cache["nc"] = _build()
    return run_bass_kernel_spmd(_cache["nc"], in_maps,
                                list(range(G * J)), trace=trace, **kw)


def kernel(inputs, h_tm1, c_tm1, basis_kernel, basis_recurrent_kernel, bias,
           _trace=False, **_kw):
    in_maps = _prep_in_maps(inputs, h_tm1, c_tm1, basis_kernel,
                            basis_recurrent_kernel, bias)
    res = _run(in_maps, trace=_trace, **_kw)
    h = np.empty((B, U), np.float32)
    c = np.empty((B, U), np.float32)
    for g in range(G):
        for j in range(J):
            r = res.results[g * J + j]
            h[g * BC:(g + 1) * BC, j * 256:(j + 1) * 256] = r["h_out"]
            c[g * BC:(g + 1) * BC, j * 256:(j + 1) * 256] = r["c_out"]
    kernel.last_results = res
    return (h, c)


# revision 26
# speedup vs baseline: 1.0393x; 1.0368x over previous
"""BasisLSTMCell Trainium2 kernel (8 NeuronCores, SPMD).

Sharding: 2-way data-parallel over batch x 4-way tensor-parallel over the
units dim.  Core c = g*4 + j handles batch rows [g*1024,(g+1)*1024) and
unit columns [j*256,(j+1)*256) of all four gates.

Math: z[b,n] = sum_{k,u} h[b,u] c[b,k] V[u,k,n]
            + sum_{k,e} x[b,e] c[b,k] W[e,k,n] + bias[n]
The contraction is laid out as r = k*1024+u (64 tiles of 128), then the
x-part r = k*96+e packed into 6 unpadded tiles (each spans two k values;
the host pre-shifts x rows so the on-device build is two aligned
partition-range multiplies), then one bias tile (row 0 = bias, against an
all-ones stationary).  On-device, hcT[r,b] = hT[u,b]*cbT[k,b] tiles are
built on VectorE from host-pretransposed bf16 inputs; TensorE accumulates
fp32 into PSUM; ScalarE+VectorE run the LSTM pointwise.

Each batch group's 8-bank accumulation is split into two 4-bank waves
with wave B lagged LAG contraction tiles behind wave A, so wave A's
pointwise epilogue overlaps the tail of the matmul stream.
"""

import sys

for _p in ("/opt/trn_rl_repo", "/root/.axon_site/_ro/trn_rl_repo"):
    if _p not in sys.path:
        sys.path.insert(0, _p)

import numpy as np
from ml_dtypes import bfloat16

import concourse.bass as bass
import concourse.mybir as mybir
import concourse.tile as tile
from concourse.bass_utils import run_bass_kernel_spmd

B, E, K, U = 2048, 96, 8, 1024
G, J = 2, 4              # batch groups x column groups (G*J = 8 cores)
BC = B // G              # 1024 batch rows per core
NC = 4 * U // J          # 1024 z-columns per core
NTT = 71                 # contraction tiles: 64 recurrent + 6 x + 1 bias
LAG = 16                 # wave-B lag in contraction tiles
BF = mybir.dt.bfloat16
F32 = mybir.dt.float32
AF = mybir.ActivationFunctionType

_cache = {}


def _split_excess_waits(nc, max_waits=1):
    """Walrus CoreV3 codegen accepts at most one sync-wait command per
    instruction; Tile's final drain can carry more.  Move the excess onto
    preceding same-engine NoOps (the engine executes in order, so the
    chain is semantically identical)."""
    for f in nc.m.functions:
        for bb in f.blocks:
            insts = list(bb.instructions)
            changed = False
            new = []
            for inst in insts:
                si = inst.sync_info
                if si is not None and len(si.on_wait) > max_waits:
                    waits = list(si.on_wait)
                    extra, keep = waits[:-max_waits], waits[-max_waits:]
                    for j in range(0, len(extra), max_waits):
                        new.append(mybir.InstNoOp(
                            name=f"{inst.name}-wsplit{j}",
                            engine=inst.engine,
                            sync_info=mybir.SyncInfo(
                                on_wait=extra[j:j + max_waits], on_update=[]),
                            bass_nofuse=True,
                        ))
                    inst.sync_info = mybir.SyncInfo(
                        on_wait=keep, on_update=si.on_update)
                    changed = True
                new.append(inst)
            if changed:
                while len(bb.instructions):
                    bb.instructions.pop()
                for i in new:
                    bb.instructions.append(i)


def _build():
    nc = bass.Bass("TRN2", target_bir_lowering=False, debug=False,
                   num_devices=G * J)
    hT_d = nc.dram_tensor("hT", [8, 128, BC], BF, kind="ExternalInput").ap()
    xTp_d = nc.dram_tensor("xTp", [6, 128, BC], BF, kind="ExternalInput").ap()
    xcb_d = nc.dram_tensor("xcb", [6, 128, BC], BF, kind="ExternalInput").ap()
    cbT_d = nc.dram_tensor("cbT", [8, 128, BC], BF, kind="ExternalInput").ap()
    Vw_d = nc.dram_tensor("Vw", [NTT, 128, NC], BF,
                          kind="ExternalInput").ap()
    cold_d = nc.dram_tensor("c_tm1", [BC, 256], F32,
                            kind="ExternalInput").ap()
    h_out_d = nc.dram_tensor("h_out", [BC, 256], F32,
                             kind="ExternalOutput").ap()
    c_out_d = nc.dram_tensor("c_out", [BC, 256], F32,
                             kind="ExternalOutput").ap()

    with tile.TileContext(nc) as tc:
        with tc.tile_pool(name="src", bufs=1) as srcp, \
             tc.tile_pool(name="w", bufs=LAG + 6) as wp, \
             tc.tile_pool(name="hc", bufs=LAG + 4) as hcp, \
             tc.tile_pool(name="pw", bufs=4) as pwp, \
             tc.tile_pool(name="io", bufs=4) as iop, \
             tc.tile_pool(name="psum", bufs=1, space="PSUM") as psp:

            ones_t = srcp.tile([128, 512], BF, tag="ones", name="ones")
            nc.vector.memset(ones_t[:, :], 1.0)

            # HAM warmup: ~6us of N=512 matmuls so the PE clock is at 8/8
            # by the time the real stream starts (shares bank q0).
            warm = psp.tile([128, 512], F32, tag="q0", name="warm")
            for _ in range(16):
                nc.tensor.matmul(warm[:, :], ones_t[:, 0:128],
                                 ones_t[:, 0:512], start=True, stop=True)

            hT_t = [None] * 8
            cb_t = [None] * 8
            # load order matches first use: t=0..7 need hT0..7 + cb0
            t_ = srcp.tile([128, BC], BF, tag="hT0", name="hT0")
            nc.sync.dma_start(out=t_[:, :], in_=hT_d[0])
            hT_t[0] = t_
            t_ = srcp.tile([128, BC], BF, tag="cb0", name="cb0")
            nc.sync.dma_start(out=t_[:, :], in_=cbT_d[0])
            cb_t[0] = t_
            for i in range(1, 8):
                t_ = srcp.tile([128, BC], BF, tag=f"hT{i}", name=f"hT{i}")
                nc.sync.dma_start(out=t_[:, :], in_=hT_d[i])
                hT_t[i] = t_
            for k in range(1, 8):
                t_ = srcp.tile([128, BC], BF, tag=f"cb{k}", name=f"cb{k}")
                nc.sync.dma_start(out=t_[:, :], in_=cbT_d[k])
                cb_t[k] = t_
            xp_t = []
            xcb_t = []
            for xt in range(6):
                t_ = srcp.tile([128, BC], BF, tag=f"xp{xt}", name=f"xp{xt}")
                nc.sync.dma_start(out=t_[:, :], in_=xTp_d[xt])
                xp_t.append(t_)
                t_ = srcp.tile([128, BC], BF, tag=f"xc{xt}", name=f"xc{xt}")
                nc.sync.dma_start(out=t_[:, :], in_=xcb_d[xt])
                xcb_t.append(t_)

            def pointwise(bank, mloc_pair, mg):
                # bank[q]: q = nh*4 + mloc; p0 = z[:,0:512] (i|f),
                # p1 = z[:,512:1024] (g|o) for m-tile mloc
                for mloc in mloc_pair:
                    row0 = mg * 512 + mloc * 128
                    p0, p1 = bank[mloc], bank[4 + mloc]
                    co = iop.tile([128, 256], F32, tag="co", name="cot")
                    nc.sync.dma_start(out=co[:, :],
                                      in_=cold_d[row0:row0 + 128, :])
                    if_s = pwp.tile([128, 512], F32, tag="if", name="ift")
                    nc.scalar.activation(if_s[:, :], p0[:, :], AF.Sigmoid)
                    g_s = pwp.tile([128, 256], F32, tag="g", name="gt")
                    nc.scalar.activation(g_s[:, :], p1[:, 0:256], AF.Tanh)
                    o_s = pwp.tile([128, 256], F32, tag="o", name="ot")
                    nc.scalar.activation(o_s[:, :], p1[:, 256:512],
                                         AF.Sigmoid)
                    t1 = pwp.tile([128, 256], F32, tag="t1", name="t1t")
                    nc.vector.tensor_mul(t1[:, :], if_s[:, 0:256], g_s[:, :])
                    t2 = pwp.tile([128, 256], F32, tag="t2", name="t2t")
                    nc.vector.tensor_mul(t2[:, :], if_s[:, 256:512],
                                         co[:, :])
                    c_n = iop.tile([128, 256], F32, tag="cn", name="cnt")
                    nc.vector.tensor_add(c_n[:, :], t1[:, :], t2[:, :])
                    th = pwp.tile([128, 256], F32, tag="th", name="tht")
                    nc.scalar.activation(th[:, :], c_n[:, :], AF.Tanh)
                    h_n = iop.tile([128, 256], F32, tag="hn", name="hnt")
                    nc.vector.tensor_mul(h_n[:, :], o_s[:, :], th[:, :])
                    nc.sync.dma_start(out=h_out_d[row0:row0 + 128, :],
                                      in_=h_n[:, :])
                    nc.sync.dma_start(out=c_out_d[row0:row0 + 128, :],
                                      in_=c_n[:, :])

            for mg in range(2):           # groups of 4 batch tiles (512 rows)
                bsl = slice(mg * 512, (mg + 1) * 512)
                bank = [psp.tile([128, 512], F32, tag=f"q{q}", name=f"q{q}")
                        for q in range(8)]
                w_tiles = {}
                hc_tiles = {}

                def mm(t, mlocs, bank=bank, w_tiles=w_tiles,
                       hc_tiles=hc_tiles):
                    hc, w_t = hc_tiles[t], w_tiles[t]
                    for mloc in mlocs:
                        for nh in range(2):
                            nc.tensor.matmul(
                                bank[nh * 4 + mloc][:, :],
                                hc[:, mloc * 128:(mloc + 1) * 128],
                                w_t[:, nh * 512:(nh + 1) * 512],
                                start=(t == 0), stop=(t == NTT - 1))

                # mg0 runs all 8 banks in lockstep (weight-DMA demand at
                # startup stays at one tile per 8 matmuls); mg1 runs the
                # lagged two-wave schedule so its wave-A pointwise overlaps
                # the matmul tail (its weights prefetch during mg0).
                lag = 0 if mg == 0 else LAG
                for s in range(NTT + lag):
                    if s < NTT:
                        t = s
                        w_t = wp.tile([128, NC], BF, tag="w", name="wt")
                        # gpsimd queue: don't serialize behind src loads
                        nc.gpsimd.dma_start(out=w_t[:, :], in_=Vw_d[t])
                        w_tiles[t] = w_t
                        if t < 64:
                            hc = hcp.tile([128, 512], BF, tag="hc",
                                          name="hct")
                            nc.vector.tensor_mul(
                                hc[:, :], hT_t[t % 8][:, bsl],
                                cb_t[t // 8][:, bsl])
                        elif t < 70:
                            xt = t - 64
                            hc = hcp.tile([128, 512], BF, tag="hc",
                                          name="hct")
                            nc.vector.tensor_mul(
                                hc[:, :], xp_t[xt][:, bsl],
                                xcb_t[xt][:, bsl])
                        else:
                            hc = ones_t
                        hc_tiles[t] = hc
                        mm(t, (0, 1, 2))       # wave A
                        if lag == 0:
                            mm(t, (3,))
                    if lag and 0 <= s - lag:
                        mm(s - lag, (3,))      # wave B
                        del hc_tiles[s - lag]
                        del w_tiles[s - lag]
                    if s == NTT - 1:
                        pointwise(bank, (0, 1, 2), mg)   # wave A epilogue
                pointwise(bank, (3,), mg)                # wave B epilogue

    _split_excess_waits(nc)
    return nc


def _prep_in_maps(inputs, h_tm1, c_tm1, basis_kernel, basis_recurrent_kernel,
                  bias):
    inputs = np.asarray(inputs, np.float32)
    x = inputs[:, :E]
    c_prob = inputs[:, E:]
    h_tm1 = np.asarray(h_tm1, np.float32)
    c_tm1 = np.asarray(c_tm1, np.float32)

    Vr = np.asarray(basis_recurrent_kernel, np.float32) \
        .transpose(1, 0, 2).reshape(K * U, 4 * U)
    Wx = np.asarray(basis_kernel, np.float32) \
        .transpose(1, 0, 2).reshape(K * E, 4 * U)
    Bt = np.zeros((128, 4 * U), np.float32)
    Bt[0] = np.asarray(bias, np.float32)
    Vw_full = np.concatenate([Vr, Wx, Bt], 0)    # [NTT*128, 4096]

    er = np.arange(K * E) % E                    # pre-shifted x row -> e
    kr = np.arange(K * E) // E                   # pre-shifted x row -> k

    in_maps = []
    for g in range(G):
        bsl = slice(g * BC, (g + 1) * BC)
        hT = np.ascontiguousarray(h_tm1[bsl].T).astype(bfloat16) \
            .reshape(8, 128, BC)
        xTp = np.ascontiguousarray(x[bsl].T[er]).astype(bfloat16) \
            .reshape(6, 128, BC)
        xcb = np.ascontiguousarray(c_prob[bsl].T[kr]).astype(bfloat16) \
            .reshape(6, 128, BC)
        cbT = np.ascontiguousarray(
            np.broadcast_to(c_prob[bsl].T[:, None, :], (8, 128, BC))
        ).astype(bfloat16)
        for j in range(J):
            cols = np.concatenate(
                [np.arange(gt * U + j * 256, gt * U + (j + 1) * 256)
                 for gt in range(4)])
            Vw_c = np.ascontiguousarray(Vw_full[:, cols]).astype(bfloat16) \
                .reshape(NTT, 128, NC)
            co = np.ascontiguousarray(c_tm1[bsl, j * 256:(j + 1) * 256])
            in_maps.append({"hT": hT, "xTp": xTp, "xcb": xcb, "cbT": cbT,
                            "Vw": Vw_c, "c_tm1": co})
    return in_maps


def _run(in_maps, trace=False, **kw):
    if "nc" not in _cache:
        _cache["nc"] = _build()
    try:
        return run_bass_kernel_spmd(_cache["nc"], in_maps,
                                    list(range(G * J)), trace=trace, **kw)
    except Exception:
        # transient device-state failures have been observed to clear on
        # the next execution; retry once without tracing
        return run_bass_kernel_spmd(_cache["nc"], in_maps,
                                    list(range(G * J)), trace=False)


def kernel(inputs, h_tm1, c_tm1, basis_kernel, basis_recurrent_kernel, bias,
           _trace=False, **_kw):
    in_maps = _prep_in_maps(inputs, h_tm1, c_tm1, basis_kernel,
                            basis_recurrent_kernel, bias)
    res = _run(in_maps, trace=_trace, **_kw)
    h = np.empty((B, U), np.float32)
    c = np.empty((B, U), np.float32)
    for g in range(G):
        for j in range(J):
            r = res.results[g * J + j]
            h[g * BC:(g + 1) * BC, j * 256:(j + 1) * 256] = r["h_out"]
            c[g * BC:(g + 1) * BC, j * 256:(j + 1) * 256] = r["c_out"]
    kernel.last_results = res
    return (h, c)


# revision 27
# speedup vs baseline: 1.0398x; 1.0005x over previous
"""BasisLSTMCell Trainium2 kernel (8 NeuronCores, SPMD).

Sharding: 2-way data-parallel over batch x 4-way tensor-parallel over the
units dim.  Core c = g*4 + j handles batch rows [g*1024,(g+1)*1024) and
unit columns [j*256,(j+1)*256) of all four gates.

Math: z[b,n] = sum_{k,u} h[b,u] c[b,k] V[u,k,n]
            + sum_{k,e} x[b,e] c[b,k] W[e,k,n] + bias[n]
The contraction is laid out as r = k*1024+u (64 tiles of 128), then the
x-part r = k*96+e packed into 6 unpadded tiles (each spans two k values;
the host pre-shifts x rows so the on-device build is two aligned
partition-range multiplies), then one bias tile (row 0 = bias, against an
all-ones stationary).  On-device, hcT[r,b] = hT[u,b]*cbT[k,b] tiles are
built on VectorE from host-pretransposed bf16 inputs; TensorE accumulates
fp32 into PSUM; ScalarE+VectorE run the LSTM pointwise.

Each batch group's 8-bank accumulation is split into two 4-bank waves
with wave B lagged LAG contraction tiles behind wave A, so wave A's
pointwise epilogue overlaps the tail of the matmul stream.
"""

import sys

for _p in ("/opt/trn_rl_repo", "/root/.axon_site/_ro/trn_rl_repo"):
    if _p not in sys.path:
        sys.path.insert(0, _p)

import numpy as np
from ml_dtypes import bfloat16

import concourse.bass as bass
import concourse.mybir as mybir
import concourse.tile as tile
from concourse.bass_utils import run_bass_kernel_spmd

B, E, K, U = 2048, 96, 8, 1024
G, J = 2, 4              # batch groups x column groups (G*J = 8 cores)
BC = B // G              # 1024 batch rows per core
NC = 4 * U // J          # 1024 z-columns per core
NTT = 71                 # contraction tiles: 64 recurrent + 6 x + 1 bias
LAG = 16                 # wave-B lag in contraction tiles
BF = mybir.dt.bfloat16
F32 = mybir.dt.float32
AF = mybir.ActivationFunctionType

_cache = {}


def _split_excess_waits(nc, max_waits=1):
    """Walrus CoreV3 codegen accepts at most one sync-wait command per
    instruction; Tile's final drain can carry more.  Move the excess onto
    preceding same-engine NoOps (the engine executes in order, so the
    chain is semantically identical)."""
    for f in nc.m.functions:
        for bb in f.blocks:
            insts = list(bb.instructions)
            changed = False
            new = []
            for inst in insts:
                si = inst.sync_info
                if si is not None and len(si.on_wait) > max_waits:
                    waits = list(si.on_wait)
                    extra, keep = waits[:-max_waits], waits[-max_waits:]
                    for j in range(0, len(extra), max_waits):
                        new.append(mybir.InstNoOp(
                            name=f"{inst.name}-wsplit{j}",
                            engine=inst.engine,
                            sync_info=mybir.SyncInfo(
                                on_wait=extra[j:j + max_waits], on_update=[]),
                            bass_nofuse=True,
                        ))
                    inst.sync_info = mybir.SyncInfo(
                        on_wait=keep, on_update=si.on_update)
                    changed = True
                new.append(inst)
            if changed:
                while len(bb.instructions):
                    bb.instructions.pop()
                for i in new:
                    bb.instructions.append(i)


def _build():
    nc = bass.Bass("TRN2", target_bir_lowering=False, debug=False,
                   num_devices=G * J)
    hT_d = nc.dram_tensor("hT", [8, 128, BC], BF, kind="ExternalInput").ap()
    xTp_d = nc.dram_tensor("xTp", [6, 128, BC], BF, kind="ExternalInput").ap()
    xcb_d = nc.dram_tensor("xcb", [6, 128, BC], BF, kind="ExternalInput").ap()
    cbT_d = nc.dram_tensor("cbT", [8, 128, BC], BF, kind="ExternalInput").ap()
    Vw_d = nc.dram_tensor("Vw", [NTT, 128, NC], BF,
                          kind="ExternalInput").ap()
    cold_d = nc.dram_tensor("c_tm1", [BC, 256], F32,
                            kind="ExternalInput").ap()
    h_out_d = nc.dram_tensor("h_out", [BC, 256], F32,
                             kind="ExternalOutput").ap()
    c_out_d = nc.dram_tensor("c_out", [BC, 256], F32,
                             kind="ExternalOutput").ap()

    with tile.TileContext(nc) as tc:
        with tc.tile_pool(name="src", bufs=1) as srcp, \
             tc.tile_pool(name="w", bufs=LAG + 6) as wp, \
             tc.tile_pool(name="hc", bufs=LAG + 4) as hcp, \
             tc.tile_pool(name="pw", bufs=4) as pwp, \
             tc.tile_pool(name="io", bufs=4) as iop, \
             tc.tile_pool(name="psum", bufs=1, space="PSUM") as psp:

            # HAM warmup: ~7us of N=512 matmuls so the PE clock is at 8/8
            # by the time the real stream starts (shares bank q0).  The
            # source is a raw, never-written SBUF tensor: the values are
            # garbage and the PSUM result is discarded (later q0 use
            # restarts accumulation with start=True), but having no
            # producer lets the warmup issue immediately at kernel start.
            warm_src = nc.alloc_sbuf_tensor("warm_src", [128, 512], BF).ap()
            warm = psp.tile([128, 512], F32, tag="q0", name="warm")
            for _ in range(20):
                nc.tensor.matmul(warm[:, :], warm_src[:, 0:128],
                                 warm_src[:, 0:512], start=True, stop=True)

            ones_t = srcp.tile([128, 512], BF, tag="ones", name="ones")
            nc.vector.memset(ones_t[:, :], 1.0)

            hT_t = [None] * 8
            cb_t = [None] * 8
            # load order matches first use: t=0..7 need hT0..7 + cb0
            t_ = srcp.tile([128, BC], BF, tag="hT0", name="hT0")
            nc.sync.dma_start(out=t_[:, :], in_=hT_d[0])
            hT_t[0] = t_
            t_ = srcp.tile([128, BC], BF, tag="cb0", name="cb0")
            nc.sync.dma_start(out=t_[:, :], in_=cbT_d[0])
            cb_t[0] = t_
            for i in range(1, 8):
                t_ = srcp.tile([128, BC], BF, tag=f"hT{i}", name=f"hT{i}")
                nc.sync.dma_start(out=t_[:, :], in_=hT_d[i])
                hT_t[i] = t_
            for k in range(1, 8):
                t_ = srcp.tile([128, BC], BF, tag=f"cb{k}", name=f"cb{k}")
                nc.sync.dma_start(out=t_[:, :], in_=cbT_d[k])
                cb_t[k] = t_
            xp_t = []
            xcb_t = []
            for xt in range(6):
                t_ = srcp.tile([128, BC], BF, tag=f"xp{xt}", name=f"xp{xt}")
                nc.sync.dma_start(out=t_[:, :], in_=xTp_d[xt])
                xp_t.append(t_)
                t_ = srcp.tile([128, BC], BF, tag=f"xc{xt}", name=f"xc{xt}")
                nc.sync.dma_start(out=t_[:, :], in_=xcb_d[xt])
                xcb_t.append(t_)

            def pointwise(bank, mloc_pair, mg):
                # bank[q]: q = nh*4 + mloc; p0 = z[:,0:512] (i|f),
                # p1 = z[:,512:1024] (g|o) for m-tile mloc
                for mloc in mloc_pair:
                    row0 = mg * 512 + mloc * 128
                    p0, p1 = bank[mloc], bank[4 + mloc]
                    co = iop.tile([128, 256], F32, tag="co", name="cot")
                    nc.sync.dma_start(out=co[:, :],
                                      in_=cold_d[row0:row0 + 128, :])
                    if_s = pwp.tile([128, 512], F32, tag="if", name="ift")
                    nc.scalar.activation(if_s[:, :], p0[:, :], AF.Sigmoid)
                    g_s = pwp.tile([128, 256], F32, tag="g", name="gt")
                    nc.scalar.activation(g_s[:, :], p1[:, 0:256], AF.Tanh)
                    o_s = pwp.tile([128, 256], F32, tag="o", name="ot")
                    nc.scalar.activation(o_s[:, :], p1[:, 256:512],
                                         AF.Sigmoid)
                    t1 = pwp.tile([128, 256], F32, tag="t1", name="t1t")
                    nc.vector.tensor_mul(t1[:, :], if_s[:, 0:256], g_s[:, :])
                    t2 = pwp.tile([128, 256], F32, tag="t2", name="t2t")
                    nc.vector.tensor_mul(t2[:, :], if_s[:, 256:512],
                                         co[:, :])
                    c_n = iop.tile([128, 256], F32, tag="cn", name="cnt")
                    nc.vector.tensor_add(c_n[:, :], t1[:, :], t2[:, :])
                    th = pwp.tile([128, 256], F32, tag="th", name="tht")
                    nc.scalar.activation(th[:, :], c_n[:, :], AF.Tanh)
                    h_n = iop.tile([128, 256], F32, tag="hn", name="hnt")
                    nc.vector.tensor_mul(h_n[:, :], o_s[:, :], th[:, :])
                    nc.sync.dma_start(out=h_out_d[row0:row0 + 128, :],
                                      in_=h_n[:, :])
                    nc.sync.dma_start(out=c_out_d[row0:row0 + 128, :],
                                      in_=c_n[:, :])

            for mg in range(2):           # groups of 4 batch tiles (512 rows)
                bsl = slice(mg * 512, (mg + 1) * 512)
                bank = [psp.tile([128, 512], F32, tag=f"q{q}", name=f"q{q}")
                        for q in range(8)]
                w_tiles = {}
                hc_tiles = {}

                def mm(t, mlocs, bank=bank, w_tiles=w_tiles,
                       hc_tiles=hc_tiles):
                    hc, w_t = hc_tiles[t], w_tiles[t]
                    for mloc in mlocs:
                        for nh in range(2):
                            nc.tensor.matmul(
                                bank[nh * 4 + mloc][:, :],
                                hc[:, mloc * 128:(mloc + 1) * 128],
                                w_t[:, nh * 512:(nh + 1) * 512],
                                start=(t == 0), stop=(t == NTT - 1))

                # mg0 runs all 8 banks in lockstep (weight-DMA demand at
                # startup stays at one tile per 8 matmuls); mg1 runs the
                # lagged two-wave schedule so its wave-A pointwise overlaps
                # the matmul tail (its weights prefetch during mg0).
                lag = 0 if mg == 0 else LAG
                for s in range(NTT + lag):
                    if s < NTT:
                        t = s
                        w_t = wp.tile([128, NC], BF, tag="w", name="wt")
                        # gpsimd queue: don't serialize behind src loads
                        nc.gpsimd.dma_start(out=w_t[:, :], in_=Vw_d[t])
                        w_tiles[t] = w_t
                        if t < 64:
                            hc = hcp.tile([128, 512], BF, tag="hc",
                                          name="hct")
                            nc.vector.tensor_mul(
                                hc[:, :], hT_t[t % 8][:, bsl],
                                cb_t[t // 8][:, bsl])
                        elif t < 70:
                            xt = t - 64
                            hc = hcp.tile([128, 512], BF, tag="hc",
                                          name="hct")
                            nc.vector.tensor_mul(
                                hc[:, :], xp_t[xt][:, bsl],
                                xcb_t[xt][:, bsl])
                        else:
                            hc = ones_t
                        hc_tiles[t] = hc
                        mm(t, (0, 1, 2))       # wave A
                        if lag == 0:
                            mm(t, (3,))
                    if lag and 0 <= s - lag:
                        mm(s - lag, (3,))      # wave B
                        del hc_tiles[s - lag]
                        del w_tiles[s - lag]
                    if s == NTT - 1:
                        pointwise(bank, (0, 1, 2), mg)   # wave A epilogue
                pointwise(bank, (3,), mg)                # wave B epilogue

    _split_excess_waits(nc)
    return nc


def _prep_in_maps(inputs, h_tm1, c_tm1, basis_kernel, basis_recurrent_kernel,
                  bias):
    inputs = np.asarray(inputs, np.float32)
    x = inputs[:, :E]
    c_prob = inputs[:, E:]
    h_tm1 = np.asarray(h_tm1, np.float32)
    c_tm1 = np.asarray(c_tm1, np.float32)

    Vr = np.asarray(basis_recurrent_kernel, np.float32) \
        .transpose(1, 0, 2).reshape(K * U, 4 * U)
    Wx = np.asarray(basis_kernel, np.float32) \
        .transpose(1, 0, 2).reshape(K * E, 4 * U)
    Bt = np.zeros((128, 4 * U), np.float32)
    Bt[0] = np.asarray(bias, np.float32)
    Vw_full = np.concatenate([Vr, Wx, Bt], 0)    # [NTT*128, 4096]

    er = np.arange(K * E) % E                    # pre-shifted x row -> e
    kr = np.arange(K * E) // E                   # pre-shifted x row -> k

    in_maps = []
    for g in range(G):
        bsl = slice(g * BC, (g + 1) * BC)
        hT = np.ascontiguousarray(h_tm1[bsl].T).astype(bfloat16) \
            .reshape(8, 128, BC)
        xTp = np.ascontiguousarray(x[bsl].T[er]).astype(bfloat16) \
            .reshape(6, 128, BC)
        xcb = np.ascontiguousarray(c_prob[bsl].T[kr]).astype(bfloat16) \
            .reshape(6, 128, BC)
        cbT = np.ascontiguousarray(
            np.broadcast_to(c_prob[bsl].T[:, None, :], (8, 128, BC))
        ).astype(bfloat16)
        for j in range(J):
            cols = np.concatenate(
                [np.arange(gt * U + j * 256, gt * U + (j + 1) * 256)
                 for gt in range(4)])
            Vw_c = np.ascontiguousarray(Vw_full[:, cols]).astype(bfloat16) \
                .reshape(NTT, 128, NC)
            co = np.ascontiguousarray(c_tm1[bsl, j * 256:(j + 1) * 256])
            in_maps.append({"hT": hT, "xTp": xTp, "xcb": xcb, "cbT": cbT,
                            "Vw": Vw_c, "c_tm1": co})
    return in_maps


def _run(in_maps, trace=False, **kw):
    if "nc" not in _cache:
        _cache["nc"] = _build()
    try:
        return run_bass_kernel_spmd(_cache["nc"], in_maps,
                                    list(range(G * J)), trace=trace, **kw)
    except Exception:
        # transient device-state failures have been observed to clear on
        # the next execution; retry once without tracing
        return run_bass_kernel_spmd(_cache["nc"], in_maps,
                                    list(range(G * J)), trace=False)


def kernel(inputs, h_tm1, c_tm1, basis_kernel, basis_recurrent_kernel, bias,
           _trace=False, **_kw):
    in_maps = _prep_in_maps(inputs, h_tm1, c_tm1, basis_kernel,
                            basis_recurrent_kernel, bias)
    res = _run(in_maps, trace=_trace, **_kw)
    h = np.empty((B, U), np.float32)
    c = np.empty((B, U), np.float32)
    for g in range(G):
        for j in range(J):
            r = res.results[g * J + j]
            h[g * BC:(g + 1) * BC, j * 256:(j + 1) * 256] = r["h_out"]
            c[g * BC:(g + 1) * BC, j * 256:(j + 1) * 256] = r["c_out"]
    kernel.last_results = res
    return (h, c)
